# revision 74
# baseline (speedup 1.0000x reference)
"""Trainium2 Bass kernel: 4D-CNN ResNet Bottleneck block, SPMD over 8 NeuronCores.

Problem (hardcoded): x[2,256,8,16,16,16] ->
  relu(bn3(conv1x1_256(relu(bn2(conv3x3x3x3(relu(bn1(conv1x1_64(x)))))))) + x)
BatchNorms use training-mode batch stats over (B,T,D,H,W).

Sharding: 16 (b,t) slices -> 2 owned t-slices/core; each core's input slab
includes the +-1 t halo (zero padded at boundaries), so no activation
exchange is needed. Three tiny AllReduces merge the BN statistics.

conv2 tap pairing: the 3x3x3x3 kernel = 81 shifted matmuls. The BN1 output
is stored in NLAYOUT padded-slab tiles [128, Y1COLS]; each tile's lower 64
partitions hold y1, the upper 64 hold y1 shifted LEFT by a per-tile flat
delta (1 = w+1, 16 = h+1/w-2, 286 = d+1/h-2/w-2), so one K=128 matmul
accumulates TWO taps.

conv2 is POSITION-MAJOR: each matmul takes a single-stride 128-cell window
of padded-flat output positions as the STATIONARY operand (walrus allows
only one free dim there) and a 64-column weight pair-block as the MOVING
operand. The cost model prices a matmul purely by moving columns, so this
runs the 128x128 array full (M=128 positions) at 64 cycles per 2 taps per
128 positions -- conv2's PE time halves vs the channel-major formulation.
Each [128pos, 64ch] PSUM tile accumulates all 41 pair-matmuls, is staged
to SBUF with a pad-mask multiply (pad cells -> exact 0), PE-transposed
back to channel-major, and lands in a padded-flat bf16 y2 [128, 41*128]
with slices on row halves. BN2 stats run as flat per-bank bn_stats over
the zero-padded columns with the [mean, E2] payload rescaled by FLAT/DHW;
the BN2 apply gathers valid cells through a strided 4D view into a compact
y2n [128, 4096], so the conv3 / Gram-BN3 machinery keeps its proven
shapes.

Schedule: conv1 (fp16, chunk-paced by an SP-queue-only DMA stream), the
BN1 apply/copies, and the conv2 tiles are software-pipelined dp-block by
dp-block (halo-slice BN1 fuses into the conv1 psum close once AR1 has
landed). BN3 stats come from a Gram matrix (y2n transposed on the PE,
G' = [y;1][y;1]^T in one PSUM bank) so conv3 needs no stats pass. All bulk
DMAs ride the SP queue -- a dma_start head-of-line blocks its issuing
engine on the exclusive HWDGE device, so scalar/vector queues carry only
compute. Dummy matmuls at BACKGROUND scheduler priority keep the tensor
engine's p-state hot through the AllReduce latency windows without ever
displacing real work; the output leaves as 8 x 1MB chunks that saturate
the DMA engines from the first conv3 epilogue tile.

Precision: conv1 runs fp16 (x is loaded once as fp16 and reused as the
residual), conv2 runs bf16 with f32 PSUM accumulation, y2 is stored bf16,
conv3 runs float32r. The residual is injected into conv3's PSUM by an
extra matmul against diag(1/bn3_scale) so the epilogue is a single pass.
Measured on HW: rel err ~3.6e-3 (gate 2e-2).
"""

import functools
import os

import numpy as np

# ---- problem constants --------------------------------------------------
B, C, T, D, H, W = 2, 256, 8, 16, 16, 16
PL = 64            # bottleneck planes
O3 = 4 * PL        # final channels (256)
NCORES = 8
EPS = 1e-5

TPC = 2                    # owned t-slices per core
SLAB = TPC + 2             # slab slices incl halo
DHW = D * H * W            # 4096
NSP_OWN = TPC * (D // 2)   # 16 owned superplanes (d-pairs)
NPOS_OWN = TPC * DHW       # 8192 positions per core
PW, PH, PD = W + 2, H + 2, D + 2
PSL = PD * PH * PW         # 5832 padded elems per slice
Y1COLS = 1 + SLAB * PSL + 1
NTAP = 81

NLAYOUT = int(os.environ.get("KERNEL_NLAYOUT", "3"))  # y1 layout tiles (1-3)
DELTAS = [1, 16, 286][:NLAYOUT]

# conv2 position-major tiling: output cells indexed by PADDED flat offset
# P = d*324 + h*18 + w within each owned slice, so every tap window is a
# single-stride column range of the L tiles (walrus: a matmul's stationary
# operand may have only ONE free dimension). Valid cells (d,h,w < 16) end
# at P = 5145; tiles of 128 rows cover [0, 5248).
VFLAT = 15 * (PH * PW) + 15 * PW + 16   # 5146
NT = (VFLAT + 127) // 128               # 41 tiles per slice
FLAT = NT * 128                         # 5248 y2 cols per slice

MM_DT = "float32r"   # conv3 matmul dtype (kept for test.py compat)
C2_DT = "bfloat16"   # conv2 matmul dtype

LAST_RESULT = None  # BassKernelResults of the most recent run (for test.py)


def _tap_dhw(tau):
    kt, r = divmod(tau, 27)
    kd, r2 = divmod(r, 9)
    kh, kw = divmod(r2, 3)
    return kt, kd, kh, kw


def _tap_flat(tau):
    kt, kd, kh, kw = _tap_dhw(tau)
    return kt * PSL + kd * (PH * PW) + kh * PW + kw


@functools.lru_cache(maxsize=2)
def _build_pairs():
    """Greedy chain pairing over odometer-ordered taps.

    Returns list of (tapA, tapB_or_None, layout_id)."""
    d2l = {d: i for i, d in enumerate(DELTAS)}
    pairs = []
    i = 0
    while i < NTAP:
        if i + 1 < NTAP:
            delta = _tap_flat(i + 1) - _tap_flat(i)
            if delta in d2l:
                pairs.append((i, i + 1, d2l[delta]))
                i += 2
                continue
        pairs.append((i, None, 0))
        i += 1
    return pairs


@functools.lru_cache(maxsize=4)
def _build(mm_dt_name=MM_DT, c2_dt_name=C2_DT, collectives=True):
    from contextlib import ExitStack

    import concourse.mybir as mybir
    import concourse.tile as tile
    from concourse import bacc

    f32 = mybir.dt.float32
    bf16 = mybir.dt.bfloat16
    fp16 = mybir.dt.float16
    f32r = mybir.dt.float32r
    AF = mybir.ActivationFunctionType
    AL = mybir.AluOpType

    pairs = _build_pairs()
    NPAIR = len(pairs)

    nc = bacc.Bacc(
        "TRN2",
        target_bir_lowering=False,
        debug=False,
        enable_asserts=False,
        num_devices=NCORES,
    )

    xsb = nc.dram_tensor("xsb", [2, 128, SLAB * DHW], fp16,
                         kind="ExternalInput").ap()
    idm = nc.dram_tensor("idm", [128, 128], fp16, kind="ExternalInput").ap()
    w1t = nc.dram_tensor("w1t", [128, 2 * PL], fp16, kind="ExternalInput").ap()
    w2t = nc.dram_tensor("w2t", [128, NPAIR * PL], bf16,
                         kind="ExternalInput").ap()
    w3t = nc.dram_tensor("w3t", [128, O3], f32, kind="ExternalInput").ap()
    gb1 = nc.dram_tensor("gb1", [64, 2], f32, kind="ExternalInput").ap()
    gb2 = nc.dram_tensor("gb2", [128, 2], f32, kind="ExternalInput").ap()
    gb3 = nc.dram_tensor("gb3", [128, 4], f32, kind="ExternalInput").ap()
    tmask = nc.dram_tensor("tmask", [64, SLAB], f32, kind="ExternalInput").ap()
    pmsk = nc.dram_tensor("pmsk", [128, NT], f32, kind="ExternalInput").ap()
    out = nc.dram_tensor("out", [2, 128, NPOS_OWN], f32, kind="ExternalOutput").ap()

    cc1_in = nc.dram_tensor("cc1_in", [64, 2], f32).ap()
    cc1_out = nc.dram_tensor("cc1_out", [64, 2], f32, addr_space="Shared").ap()
    cc2_in = nc.dram_tensor("cc2_in", [64, 4], f32).ap()
    cc2_out = nc.dram_tensor("cc2_out", [64, 4], f32, addr_space="Shared").ap()
    cc3_in = nc.dram_tensor("cc3_in", [128, 4], f32).ap()
    cc3_out = nc.dram_tensor("cc3_out", [128, 4], f32, addr_space="Shared").ap()
    RG = [list(range(NCORES))]

    def allreduce(cin, cout, q=None):
        if collectives:
            nc.gpsimd.collective_compute(
                "AllReduce", AL.add, replica_groups=RG,
                ins=[cin], outs=[cout],
            )
        else:  # timing-sim variant: stand-in DMA with the same deps
            (q or nc.gpsimd).dma_start(out=cout, in_=cin)

    with tile.TileContext(nc) as tc, ExitStack() as st:
        const = st.enter_context(tc.tile_pool(name="const", bufs=1))
        smalls = st.enter_context(tc.tile_pool(name="smalls", bufs=1))

        def sm(shape, nm):
            return smalls.tile(shape, f32, tag=nm, name=nm)

        # ---- persistent SBUF tensors ---------------------------------
        w1sb = const.tile([128, 2 * PL], fp16, tag="w1sb", name="w1sb")
        w2sb = const.tile([128, NPAIR * PL], bf16, tag="w2sb", name="w2sb")
        w3sb = const.tile([128, O3], f32r, tag="w3sb", name="w3sb")
        w3bf = const.tile([64, O3], bf16, tag="w3bf", name="w3bf")
        w3stg = const.tile([128, O3], f32, tag="w3stg", name="w3stg")
        idr = const.tile([128, 64], f32r, tag="idr", name="idr")
        idr128 = const.tile([128, 128], f32r, tag="idr128", name="idr128")
        ones64 = const.tile([64, 1], bf16, tag="ones64", name="ones64")
        idmsb = const.tile([128, 128], fp16, tag="idmsb", name="idmsb")
        diag3 = const.tile([128, 256], fp16, tag="diag3", name="diag3")
        gb1sb = smalls.tile([64, 2], f32, tag="gb1sb", name="gb1sb")
        gb2sb = sm([128, 2], "gb2sb")
        gb3sb = sm([128, 4], "gb3sb")
        tmsb = smalls.tile([64, SLAB], f32, tag="tmsb", name="tmsb")
        pmsb = smalls.tile([128, NT], f32, tag="pmsb", name="pmsb")
        # p-state warm-up signal: AR-window pin-copies write a few columns
        # so the dummy matmuls (lhsT=wsig) cannot be scheduled before the
        # window opens
        wsig = smalls.tile([128, PL], fp16, tag="wsig", name="wsig")
        # x kept resident: conv1 rhs for owned superplanes + conv3 residual
        x_own = [const.tile([128, NPOS_OWN], fp16, tag=f"xo{cb}", name=f"xo{cb}")
                 for cb in range(2)]
        # y2: padded-flat, slice-major rows (0-63 slice0 / 64-127 slice1)
        y2 = const.tile([128, FLAT], bf16, tag="y2", name="y2")
        st1 = smalls.tile([64, NSP_OWN * 6], f32, tag="st1", name="st1")
        st2 = sm([128, 11 * 6], "st2")

        # w1 + owned x first so conv1 starts ASAP. The x stream rides the
        # two HWDGE queues (sync + scalar) whose dispatch is free for the
        # compute engines; gpsimd DMAs cost ~1us of gpsimd engine time each
        # and are reserved for the tiny AllReduce chains.
        nc.sync.dma_start(out=w1sb[:], in_=w1t[:])
        # owned x in consumption order (so-major within each dp-quad); the
        # first pair goes as 1024-col halves so conv1 starts ~4us earlier.
        # HWDGE descriptor generation costs ~630ns per DMA instruction, so
        # the rest goes as 2048-col chunks.
        # All bulk DMAs ride the SP queue: a dma_start occupies its issuing
        # engine's queue while it waits for the (exclusive) HWDGE device, so
        # putting them on scalar/vector queues head-of-line blocks the
        # compute those engines owe the pipeline.
        first = True
        for dpq in range(2):
            for so in range(TPC):
                off = so * DHW + dpq * 2048
                for cb in range(2):
                    if first:
                        # split for smoother arrival vs conv1 burn rate
                        nc.sync.dma_start(
                            out=x_own[cb][:, off:off + 512],
                            in_=xsb[cb, :, DHW + off:DHW + off + 512])
                        nc.sync.dma_start(
                            out=x_own[cb][:, off + 512:off + 1024],
                            in_=xsb[cb, :,
                                    DHW + off + 512:DHW + off + 1024])
                        nc.sync.dma_start(
                            out=x_own[cb][:, off + 1024:off + 2048],
                            in_=xsb[cb, :,
                                    DHW + off + 1024:DHW + off + 2048])
                    else:
                        for hh in range(2):
                            o2 = off + 1024 * hh
                            nc.sync.dma_start(
                                out=x_own[cb][:, o2:o2 + 1024],
                                in_=xsb[cb, :, DHW + o2:DHW + o2 + 1024])
                if first:
                    nc.sync.dma_start(out=idmsb[:], in_=idm[:])
                    first = False
            if dpq == 0:
                nc.sync.dma_start(out=gb1sb[:], in_=gb1[:])
                nc.sync.dma_start(out=tmsb[:], in_=tmask[:])
                nc.sync.dma_start(out=pmsb[:], in_=pmsk[:])
        nc.vector.memset(ones64[:], 1.0)
        nc.gpsimd.memset(wsig[:], 1.0)


        # ---- BN finalize helpers -------------------------------------
        def bn_reduce_prep(mv, arin, nm):
            """mv[:,0]=mean, mv[:,1]=var -> arin[:,0]=mean, arin[:,1]=E[x^2]."""
            t = sm([mv.shape[0], 1], f"bnp_{nm}")
            nc.vector.tensor_tensor(out=t[:], in0=mv[:, 0:1], in1=mv[:, 0:1],
                                    op=AL.mult)
            nc.vector.tensor_tensor(out=arin[:, 1:2], in0=mv[:, 1:2], in1=t[:],
                                    op=AL.add)
            nc.vector.tensor_copy(out=arin[:, 0:1], in_=mv[:, 0:1])

        def bn_finalize(msum, e2sum, inv_n, g_ap, b_ap, scale, bias, nm):
            """msum=sum(mean_l), e2sum=sum(E2_l) -> scale/bias (same shape)."""
            P = scale.shape[0]
            ncol = scale.shape[1]
            nm_ = sm([P, ncol], f"nm_{nm}")
            tt = sm([P, ncol], f"tt_{nm}")
            e2 = sm([P, ncol], f"e2_{nm}")
            rs = sm([P, ncol], f"rs_{nm}")
            nc.vector.tensor_scalar_mul(nm_[:], msum, -inv_n)
            nc.vector.tensor_tensor(out=tt[:], in0=nm_[:], in1=nm_[:],
                                    op=AL.mult)
            nc.vector.scalar_tensor_tensor(
                out=e2[:], in0=e2sum, scalar=inv_n, in1=tt[:],
                op0=AL.mult, op1=AL.subtract)
            nc.vector.tensor_scalar_add(e2[:], e2[:], EPS)
            nc.vector.reciprocal(out=tt[:], in_=e2[:])
            nc.scalar.activation(rs[:], tt[:], AF.Sqrt)
            nc.vector.tensor_tensor(out=scale[:], in0=g_ap, in1=rs[:],
                                    op=AL.mult)
            nc.vector.tensor_tensor(out=tt[:], in0=nm_[:], in1=scale[:],
                                    op=AL.mult)
            nc.vector.tensor_tensor(out=bias[:], in0=tt[:], in1=b_ap,
                                    op=AL.add)

        scale1 = smalls.tile([64, 1], f32, tag="scale1", name="scale1")
        bias1 = smalls.tile([64, 1], f32, tag="bias1", name="bias1")
        sc_all = smalls.tile([64, SLAB], f32, tag="sc_all", name="sc_all")
        bi_all = smalls.tile([64, SLAB], f32, tag="bi_all", name="bi_all")
        scale3 = sm([128, 2], "scale3")
        bias3 = sm([128, 2], "bias3")

        with tc.tile_pool(name="Lp", bufs=1) as Lpool:
            Lt = [Lpool.tile([128, Y1COLS], bf16, tag=f"L{i}", name=f"L{i}")
                  for i in range(NLAYOUT)]
            # Zero only L0's padding borders (+margins) — the flat block
            # copies below propagate them to every other layout half.
            # Split across gpsimd/DVE so it finishes during the conv1 DMAs.
            L0v = Lt[0][:, 1:1 + SLAB * PSL].rearrange(
                "p (s d h w) -> p s d h w", s=SLAB, d=PD, h=PH, w=PW)
            nc.gpsimd.memset(Lt[0][:, 0:1], 0.0)
            for li, Lx in enumerate(Lt):
                # tail cells are read via zero-weight upper halves; they are
                # never copy-written, so zero them to keep NaN out of PSUM
                nc.gpsimd.memset(Lx[:, Y1COLS - 2:Y1COLS], 0.0)
                # the delta-shifted upper copies end at col Y1COLS-1-delta;
                # flat windows read past that on pad rows, so zero the gap
                if DELTAS[li] > 1:
                    nc.gpsimd.memset(
                        Lx[64:128, Y1COLS - 1 - DELTAS[li]:Y1COLS - 2], 0.0)
            for s in (1, 2, 3, 0):  # owned slices first: conv1 writes them
                eng = nc.gpsimd if s in (1, 0) else nc.vector
                eng.memset(L0v[:, s, 0], 0.0)               # d=0 plane
                eng.memset(L0v[:, s, PD - 1], 0.0)          # d=17 plane
                eng.memset(L0v[:, s, 1:PD - 1, 0], 0.0)     # h=0 rows
                eng.memset(L0v[:, s, 1:PD - 1, PH - 1], 0.0)
                eng.memset(L0v[:, s, 1:PD - 1, 1:PH - 1, 0], 0.0)   # w borders
                eng.memset(L0v[:, s, 1:PD - 1, 1:PH - 1, PW - 1], 0.0)

            # full-partition 5D views (margin offset 1) for conv2 rhs
            L5 = [Lx[:, 1:1 + SLAB * PSL].rearrange(
                "p (s d h w) -> p s d h w", s=SLAB, d=PD, h=PH, w=PW)
                for Lx in Lt]

            def win64(lx, plo, base):
                """[64, 2, 16, 16] strided window at flat col `base`."""
                v = lx[plo:plo + 64, base:base + 2 * (PH * PW)].rearrange(
                    "p (d x) -> p d x", d=2, x=PH * PW)
                v = v[:, :, :16 * PW].rearrange(
                    "p d (h w) -> p d h w", h=16, w=PW)
                return v[:, :, :, 0:16]

            def vbase(s, dp):
                """Flat col of valid-block origin (d=2dp+1,h=1,w=1), lower."""
                return 1 + s * PSL + (2 * dp + 1) * (PH * PW) + PW + 1

            def blk_cols(s, dp):
                # dp-block padded extent (full d-planes, flat-contiguous);
                # dp 0/7 absorb the d=0/17 border planes
                d0 = 2 * dp + 1 if dp > 0 else 0
                d1 = 2 * dp + 3 if dp < D // 2 - 1 else PD
                return (1 + s * PSL + d0 * (PH * PW),
                        1 + s * PSL + d1 * (PH * PW))

            # ====== conv1 + flipped conv2, software-pipelined ==========
            # conv1 computes each superplane into a [64,512] PSUM tile,
            # raw-copies the bf16 result into L0's lower half, and collects
            # BN1 stats for owned superplanes (so the stats AllReduce issues
            # as early as possible). Halo conv1 (with BN1+relu fused into
            # the psum close), the shifted-layout copies for ALL layouts,
            # and conv2 are then pipelined dp-block by dp-block.
            #
            # conv2 is position-major: each matmul takes the y1 window as
            # the STATIONARY operand ([128, 8h, 16w] = 128 positions) and a
            # 64-column weight pair-block as the MOVING operand, i.e. 64
            # moving columns per 2 taps per 128 positions. The cost model
            # prices a matmul by moving columns only, so this runs the
            # 128x128 array full (vs 64 rows in the channel-major
            # formulation) -- conv2's PE time halves to the bf16 roofline.
            # Each [128pos, 64ch] PSUM tile accumulates all NPAIR
            # pair-matmuls, is staged to SBUF (f32r), transposed back to
            # channel-major on the PE, and lands in y2 with exactly the
            # baseline layout, so conv3/BN3 machinery is untouched.
            cp_flip = [0]
            cp_pool_ok = [False]

            def cpy(dst, src_):
                # rotate DVE/ACT/DVE/Pool to balance engine load; keep the
                # slow Pool engine out of the startup-critical first blocks
                cp_flip[0] = (cp_flip[0] + 1) % 4
                if cp_flip[0] == 0 and cp_pool_ok[0]:
                    nc.gpsimd.tensor_copy(out=dst, in_=src_)
                elif cp_flip[0] == 2:
                    nc.scalar.copy(out=dst, in_=src_)
                else:
                    nc.vector.tensor_copy(out=dst, in_=src_)

            with tc.tile_pool(name="psr", bufs=3, space="PSUM") as psr, \
                 tc.tile_pool(name="c2p", bufs=3, space="PSUM") as c2p, \
                 tc.tile_pool(name="pTp", bufs=2, space="PSUM") as pTp, \
                 tc.tile_pool(name="stgp", bufs=8) as stgp, \
                 tc.tile_pool(name="xh", bufs=4) as xh:

                hchunk = {}

                def load_halo(dp):
                    # halo x as [128,1024] dp-pair chunks (SP queue)
                    for s in (0, 3):
                        for cb in range(2):
                            tr = xh.tile([128, 1024], fp16, tag="xhr",
                                         name="xhr")
                            c0 = s * DHW + dp * 512
                            nc.sync.dma_start(out=tr[:],
                                              in_=xsb[cb, :, c0:c0 + 1024])
                            hchunk[(s, cb)] = tr

                # dp0 halo rides right behind the owned stream so the AR1
                # latency window has halo conv1 work for the PE; conv2
                # weights and the endgame consts land after it
                load_halo(0)
                nc.sync.dma_start(out=w2sb[:], in_=w2t[:])
                nc.sync.dma_start(out=w3stg[:], in_=w3t[:])
                nc.vector.tensor_copy(out=idr128[:], in_=idmsb[:])

                def conv1_sp(s, dp, rhs_cb, stats_i, fuse_bn=False):
                    ps = psr.tile([64, 512], f32, tag="c1p")
                    for cb in range(2):
                        nc.tensor.matmul(
                            ps[:],
                            lhsT=w1sb[:, cb * PL:(cb + 1) * PL],
                            rhs=rhs_cb[cb],
                            start=(cb == 0), stop=(cb == 1),
                        )
                    if stats_i is not None:
                        nc.vector.bn_stats(
                            out=st1[:, stats_i * 6:(stats_i + 1) * 6],
                            in_=ps[:])
                    dst = win64(Lt[0], 0, vbase(s, dp))
                    src = ps.rearrange("p (d h w) -> p d h w",
                                       d=2, h=16, w=16)
                    if fuse_bn:
                        # halo slices run post-AR1: BN1+relu applied right
                        # out of PSUM (masked scale/bias zero t-borders)
                        nc.scalar.activation(dst, src, AF.Relu,
                                             bias=bi_all[:, s:s + 1],
                                             scale=sc_all[:, s:s + 1])
                    elif stats_i is None:
                        # unfused halo close: DVE, so the ACT queue stays
                        # clear for the AR1-gated BN1 applies
                        nc.vector.tensor_copy(out=dst, in_=src)
                    else:
                        nc.scalar.copy(out=dst, in_=src)

                # owned first (slices 1,2), chunk-major = arrival order
                for dpq in range(2):
                    for so in range(TPC):
                        for dp in range(4 * dpq, 4 * dpq + 4):
                            off = so * DHW + dp * 512
                            conv1_sp(
                                so + 1, dp,
                                [x_own[cb][:, off:off + 512]
                                 for cb in range(2)],
                                so * (D // 2) + dp)

                mv1 = sm([64, 2], "mv1")
                arin1 = sm([64, 2], "arin1")
                nc.vector.bn_aggr(out=mv1[:], in_=st1[:])
                bn_reduce_prep(mv1, arin1, "bn1")
                nc.gpsimd.dma_start(out=cc1_in[:], in_=arin1[:])
                allreduce(cc1_in[:], cc1_out[:])
                g1s = sm([64, 2], "g1s")
                nc.gpsimd.dma_start(out=g1s[:], in_=cc1_out[:])
                bn_finalize(g1s[:, 0:1], g1s[:, 1:2], 1.0 / NCORES,
                            gb1sb[:, 0:1], gb1sb[:, 1:2],
                            scale1, bias1, "bn1")
                # per-slab-slice masked scale/bias (zero invalid slices)
                nc.vector.tensor_scalar_mul(sc_all[:], tmsb[:], scale1[:])
                nc.vector.tensor_scalar_mul(bi_all[:], tmsb[:], bias1[:])
                # keep the PE p-state hot through the AR1 latency tail
                # (lhsT=wsig pins these after the stats are in flight)
                nc.vector.tensor_copy(out=wsig[0:64, 0:2], in_=arin1[:])
                with tc.high_priority(offset=-(1 << 22)):
                    for _ in range(16):
                        pwm = psr.tile([64, 512], f32, tag="c1p", name="pwm")
                        nc.tensor.matmul(pwm[:], lhsT=wsig[:],
                                         rhs=x_own[0][:, 0:512],
                                         start=True, stop=True)

                def do_dp(dp):
                    # halo conv1 (BN1 fused), owned BN1 applies, and every
                    # layout copy for this dp-block
                    cp_pool_ok[0] = dp >= 2
                    if dp % 2 == 0 and dp > 0:
                        load_halo(dp)
                    # blocks 0-1 run inside the AR1 latency window: their
                    # halo closes must not depend on the reduced stats, so
                    # they take the copy-then-apply path; later blocks fuse
                    # BN1+relu into the psum close.
                    fuse = dp >= 6
                    for s in (0, 3):
                        lo_ = (dp % 2) * 512
                        conv1_sp(s, dp,
                                 [hchunk[(s, cb)][:, lo_:lo_ + 512]
                                  for cb in range(2)], None, fuse_bn=fuse)
                    for s in ((1, 2) if fuse else (0, 1, 2, 3)):
                        bs = vbase(s, dp)
                        dst = win64(Lt[0], 0, bs)
                        if (s + dp) % 2:
                            nc.scalar.activation(
                                dst, dst, AF.Relu,
                                bias=bi_all[:, s:s + 1],
                                scale=sc_all[:, s:s + 1])
                        else:
                            nc.vector.tensor_scalar(
                                dst, dst, sc_all[:, s:s + 1],
                                bi_all[:, s:s + 1],
                                op0=AL.mult, op1=AL.add)
                            nc.vector.tensor_scalar_max(dst, dst, 0.0)
                    for s in range(SLAB):
                        lo, hi = blk_cols(s, dp)
                        cpy(Lt[0][64:128, lo - 1:hi - 1], Lt[0][0:64, lo:hi])
                        for li, dlt in enumerate(DELTAS):
                            if li == 0:
                                continue
                            cpy(Lt[li][0:64, lo:hi], Lt[0][0:64, lo:hi])
                            crop = max(0, dlt - lo)
                            cpy(Lt[li][64:128, lo + crop - dlt:hi - dlt],
                                Lt[0][0:64, lo + crop:hi])

                # ---- flat position-major conv2 tiles -----------------
                # The stage copy multiplies by a per-row pad mask, so y2
                # pad cells are exact zeros. BN2 stats then run as flat
                # per-bank bn_stats (walrus: 1 free dim, 6-col output) and
                # the [mean, E2] payload is rescaled by FLAT/DHW.
                NSB = (FLAT + 511) // 512          # 11 stats banks
                MLAST = VFLAT - 128 * (NT - 1)

                def c2_tile(s, i):
                    b, q = divmod(i, 4)
                    M = 128 if i < NT - 1 else MLAST
                    if q == 0:
                        cstate["c2"] = c2p.tile([128, 512], f32, tag="c2",
                                                name="c2b")
                        cstate["pT"] = pTp.tile([64, 512], f32r, tag="pT",
                                                name="pTb")
                    ps = cstate["c2"][0:M, q * PL:(q + 1) * PL]
                    C0 = 128 * i
                    for j, (ta, tb, lid) in enumerate(pairs):
                        kt, kd, kh, kw = _tap_dhw(ta)
                        base = (1 + (s + kt) * PSL + kd * (PH * PW)
                                + kh * PW + kw + C0)
                        nc.tensor.matmul(
                            ps, lhsT=Lt[lid][:, base:base + M],
                            rhs=w2sb[:, j * PL:(j + 1) * PL],
                            start=(j == 0), stop=(j == NPAIR - 1))
                    stg = stgp.tile([128, PL], f32r, tag="stg", name="stg")
                    if M < 128:
                        # partial tile: zero first (partition slices must be
                        # 32-aligned), then overwrite the valid rows
                        nc.vector.memset(stg[:].bitcast(f32), 0.0)
                    nc.vector.tensor_scalar_mul(stg[0:M, :], ps,
                                                pmsb[0:M, i:i + 1])
                    nc.tensor.transpose(
                        out=cstate["pT"][:, q * 128:(q + 1) * 128],
                        in_=stg[:], identity=idr128[:])
                    if q == 3 or i == NT - 1:
                        ncol = 128 * (q + 1)
                        ydst = y2[s * 64:(s + 1) * 64, 512 * b:512 * b + ncol]
                        cq_flip[0] ^= 1
                        if cq_flip[0]:
                            nc.scalar.copy(out=ydst,
                                           in_=cstate["pT"][:, 0:ncol])
                        else:
                            nc.vector.tensor_copy(out=ydst,
                                                  in_=cstate["pT"][:, 0:ncol])
                        if s == 1:
                            hi2 = min(512 * b + 512, FLAT)
                            nc.vector.bn_stats(
                                out=st2[:, b * 6:(b + 1) * 6],
                                in_=y2[:, 512 * b:hi2])

                cstate = {}
                cq_flip = [0]
                done = 0
                for i in range(NT):
                    # gate each tile on the dp-blocks whose L copies its
                    # windows read (incl. the delta-shifted upper halves)
                    M = 128 if i < NT - 1 else MLAST
                    pmax = min(PD - 1, (128 * i + M - 1 + 686) // (PH * PW))
                    need = min(D // 2 - 1, max(0, (pmax - 1) // 2))
                    while done <= need:
                        do_dp(done)
                        done += 1
                    c2_tile(0, i)
                while done < D // 2:
                    do_dp(done)
                    done += 1
                for i in range(NT):
                    c2_tile(1, i)

                # bn2/bn3 gamma/beta ride behind the halo stream
                nc.sync.dma_start(out=gb2sb[:], in_=gb2[:])
                nc.sync.dma_start(out=gb3sb[:], in_=gb3[:])
                nc.vector.tensor_copy(out=w3sb[:], in_=w3stg[:])
                nc.vector.tensor_copy(out=w3bf[:], in_=w3stg[0:64, :])
                nc.vector.tensor_copy(out=idr[0:64, :], in_=idmsb[0:64, 0:64])
                nc.vector.tensor_copy(out=idr[64:128, :],
                                      in_=idmsb[64:128, 64:128])

        # PE warm-up: the tensor engine drops to its mid p-state (2x
        # slower) after ~any idle and needs 3us of continuous work to
        # re-ramp. During the AR2/AR3 latency windows the PE has nothing
        # real to do, so it chews dependency-free dummy matmuls into a
        # scratch bank, keeping the clock at full speed for the endgame.
        psw_ctx = tc.tile_pool(name="psw", bufs=1, space="PSUM")
        psw = psw_ctx.__enter__()
        wt = psw.tile([64, 512], f32, tag="warm", name="warm")

        def warm(n):
            # background priority: the ready-heap only picks these when no
            # real PE work is runnable, so oversizing is harmless (leftovers
            # drain into the DMA-bound tail)
            with tc.high_priority(offset=-(1 << 22)):
                for _ in range(n):
                    nc.tensor.matmul(wt[:], lhsT=wsig[:],
                                     rhs=x_own[0][:, 0:512],
                                     start=True, stop=True)

        # L tiles freed here.
        # Both y2 partition halves hold the same channels (position split),
        # so their stats ride the collective as separate COLUMNS [64,4] and
        # merge with a same-partition column add afterwards; the finalized
        # scale/bias then broadcasts back to the upper half with one DMA.
        # The chain DMAs use the sync HWDGE queue (idle and dispatch-free
        # at this point) — gpsimd DMAs cost ~1us of engine time each.
        mv2 = sm([128, 2], "mv2")
        arin2 = sm([128, 2], "arin2")
        # pin the AR2 warm fill to the LAST stats entry so it starts the
        # moment conv2 drains (values are junk; only the dep matters)
        nc.vector.tensor_copy(out=wsig[:, 2:4], in_=st2[:, 58:60])
        nc.vector.bn_aggr(out=mv2[:], in_=st2[:])
        bn_reduce_prep(mv2, arin2, "bn2")
        nc.vector.tensor_scalar_mul(arin2[:], arin2[:], FLAT / DHW)
        nc.sync.dma_start(out=cc2_in[:, 0:2], in_=arin2[0:64, :])
        nc.sync.dma_start(out=cc2_in[:, 2:4], in_=arin2[64:128, :])
        allreduce(cc2_in[:], cc2_out[:], q=nc.sync)
        g2s = sm([128, 4], "g2s")
        fs2 = sm([128, 2], "fs2")
        sb2 = sm([128, 2], "sb2")
        nc.sync.dma_start(out=g2s[0:64, :], in_=cc2_out[:])
        nc.sync.dma_start(out=g2s[64:128, :], in_=cc2_out[:])
        warm(58)
        nc.vector.tensor_tensor(out=fs2[:], in0=g2s[:, 0:2], in1=g2s[:, 2:4],
                                op=AL.add)
        bn_finalize(fs2[:, 0:1], fs2[:, 1:2], 1.0 / (2 * NCORES),
                    gb2sb[:, 0:1], gb2sb[:, 1:2],
                    sb2[:, 0:1], sb2[:, 1:2], "bn2")
        scale2, bias2 = sb2[:, 0:1], sb2[:, 1:2]

        with tc.tile_pool(name="zp", bufs=1) as zpool:
            # y2n is COMPACT [128, 4096]: the BN2 apply gathers the valid
            # cells out of the padded-flat y2 via a strided 4D source view,
            # so the yT/Gram/conv3 machinery keeps the baseline's shapes.
            y2n = zpool.tile([128, NSP_OWN * 256], f32r, tag="y2n",
                             name="y2n")
            y2vv = y2[:, 0:16 * PH * PW].rearrange(
                "p (d h w) -> p d h w", d=16, h=PH, w=PW)
            for g in range(8):  # BN2 + relu (rounds to f32r), ACT-heavy
                ydst = y2n[:, g * 512:(g + 1) * 512]
                ysrc = y2vv[:, 2 * g:2 * g + 2, 0:16, 0:16]
                if g % 2 == 0:
                    nc.scalar.activation(ydst, ysrc, AF.Relu,
                                         bias=bias2[:], scale=scale2[:])
                else:
                    nc.vector.tensor_scalar(ydst, ysrc, scale2[:], bias2[:],
                                            op0=AL.mult, op1=AL.add)
                    nc.vector.tensor_scalar_max(ydst, ydst, 0.0)

            def c3_mm(ps4, sl, dp, oh, stop=True):
                rhs = y2n[sl * 64:(sl + 1) * 64,
                          dp * 512:(dp + 1) * 512].bitcast(f32r)
                pg = ps4.tile([128, 512], f32, tag="c3")
                nc.tensor.matmul(
                    pg[:],
                    lhsT=w3sb[sl * 64:(sl + 1) * 64,
                              oh * 128:(oh + 1) * 128].bitcast(f32r),
                    rhs=rhs, start=True, stop=stop)
                return pg

            # ======== conv3 pass 1: BN3 stats via Gram matrix ==========
            # sum_pos(y3) = w3^T s and sum_pos(y3^2)_o = w3_o^T G w3_o with
            # s = sum_pos(y2n), G = y2n y2n^T. y2n is transposed on the PE
            # (64 chunks of [64,128]), augmented with a ones column, and
            # G' = [[G, s], [s^T, N]] accumulates in one PSUM bank. This
            # replaces 32 full-width matmuls + 21us of DVE bn_stats.
            NCH = 64  # transposed y2n chunks of [64, 128]
            yT = zpool.tile([128, NCH * 65], bf16, tag="yT", name="yT")
            ones1f = smalls.tile([1, 1], f32, tag="ones1f", name="ones1f")
            nc.vector.memset(ones1f[:], 1.0)
            nc.vector.memset(
                yT.rearrange("p (c e) -> p c e", c=NCH, e=65)[:, :, 64], 1.0)
            with tc.tile_pool(name="psT", bufs=2, space="PSUM") as psT, \
                 tc.tile_pool(name="psG", bufs=1, space="PSUM") as psGp, \
                 tc.tile_pool(name="psS", bufs=1, space="PSUM") as psSp:
                psG = psGp.tile([65, 65], f32, tag="gacc", name="psG")
                psSb = psSp.tile([128, 512], f32, tag="psSb", name="psSb")
                for b in range((NCH + 7) // 8):
                    ng = min(8, NCH - 8 * b)
                    pt = psT.tile([128, 512], f32r, tag="ptr", name="pt")
                    for j in range(ng):
                        half, ci = divmod(8 * b + j, 32)
                        nc.tensor.transpose(
                            out=pt[:, j * 64:(j + 1) * 64],
                            in_=y2n[half * 64:(half + 1) * 64,
                                    ci * 128:(ci + 1) * 128].bitcast(f32r),
                            identity=idr[half * 64:(half + 1) * 64, :])
                    dst = yT[:, b * 520:b * 520 + ng * 65].rearrange(
                        "p (c e) -> p c e", c=ng, e=65)[:, :, 0:64]
                    eng = nc.vector.tensor_copy if b % 2 else nc.scalar.copy
                    eng(out=dst, in_=pt[:, 0:ng * 64].rearrange(
                        "p (c e) -> p c e", c=ng, e=64))
                for c in range(NCH):
                    ch = yT[:, c * 65:(c + 1) * 65]
                    nc.tensor.matmul(psG[:], lhsT=ch, rhs=ch,
                                     start=(c == 0), stop=(c == NCH - 1))
                gsb = zpool.tile([65, 65], bf16, tag="gsb", name="gsb")
                nc.vector.tensor_copy(out=gsb[:], in_=psG[:])
                m1p = psSb[0:64, 0:256]
                nc.tensor.matmul(m1p[:], lhsT=gsb[0:64, 0:64], rhs=w3bf[:],
                                 start=True, stop=True)
                m1sb = zpool.tile([64, 256], bf16, tag="m1sb", name="m1sb")
                prod = zpool.tile([64, 256], bf16, tag="prod", name="prod")
                nc.vector.tensor_copy(out=m1sb[:], in_=m1p[:])
                nc.vector.tensor_tensor(out=prod[:], in0=w3bf[:],
                                        in1=m1sb[:], op=AL.mult)
                e2p = psSb[0:1, 256:512]
                nc.tensor.matmul(e2p[:], lhsT=ones64[:], rhs=prod[:],
                                 start=True, stop=True)
                e2sb = zpool.tile([1, 256], f32, tag="e2sb", name="e2sb")
                nc.scalar.copy(out=e2sb[:], in_=e2p[:])
                arin3 = sm([128, 4], "arin3")
                for oh in range(2):
                    mp = psSb[:, 504 + oh:505 + oh]
                    nc.tensor.matmul(
                        mp[:], lhsT=w3bf[:, oh * 128:(oh + 1) * 128],
                        rhs=gsb[0:64, 64:65], start=True, stop=True)
                    nc.vector.tensor_copy(out=arin3[:, 2 * oh:2 * oh + 1],
                                          in_=mp[:])
                    ep = psSb[:, 508 + oh:509 + oh]
                    nc.tensor.transpose(
                        out=ep[:], in_=e2sb[0:1, oh * 128:(oh + 1) * 128],
                        identity=ones1f[:])
                    nc.vector.tensor_copy(
                        out=arin3[:, 2 * oh + 1:2 * oh + 2], in_=ep[:])
            nc.vector.tensor_copy(out=wsig[:, 4:8], in_=arin3[:])
            nc.sync.dma_start(out=cc3_in[:], in_=arin3[:])
            allreduce(cc3_in[:], cc3_out[:], q=nc.sync)
            # bridge the AR3 round-trip + finalize chain (~8us) so the PE
            # stays at full p-state into conv3
            warm(40)
            g3s = sm([128, 4], "g3s")
            nc.sync.dma_start(out=g3s[:], in_=cc3_out[:])
            g3v = g3s.rearrange("p (o two) -> p o two", o=2, two=2)
            bn_finalize(g3v[:, :, 0], g3v[:, :, 1],
                        1.0 / (NCORES * NPOS_OWN),
                        gb3sb[:, 0:2], gb3sb[:, 2:4], scale3, bias3, "bn3")
            recip3 = sm([128, 2], "recip3")
            nc.vector.reciprocal(out=recip3[:], in_=scale3[:])
            for oh in range(2):
                # diag(1/scale3): lets the PE inject the residual into PSUM
                nc.vector.tensor_scalar_mul(
                    diag3[:, oh * 128:(oh + 1) * 128], idmsb[:],
                    recip3[:, oh:oh + 1])

            # ==== conv3 pass 2 + fused BN3/residual/relu/store ========
            # The c3 matmul of tile i+8 is issued before tile i's diag
            # injection, so 8 PSUM banks of c3 work run during the AR3
            # latency and the PE never sits behind the scale3 chain.
            # Epilogues land in contiguous 4-superplane staging buffers so
            # the output leaves as 8 big DMAs instead of 32 small ones.
            with tc.tile_pool(name="ps5", bufs=7, space="PSUM") as ps5, \
                 tc.tile_pool(name="fino", bufs=3) as fino:
                tiles = [(oh, sp) for oh in range(2) for sp in range(16)]
                pend = []
                chunk = [None]

                def finish(pg, oh, sp):
                    nc.tensor.matmul(
                        pg[:], lhsT=diag3[:, oh * 128:(oh + 1) * 128],
                        rhs=x_own[oh][:, sp * 512:(sp + 1) * 512],
                        start=False, stop=True)
                    # the first chunk of oh0 ships as two 2-tile pieces so
                    # the bandwidth-bound output stream starts earlier
                    csz = 2 if (oh == 0 and sp < 4) else 4
                    if sp % csz == 0:
                        chunk[0] = fino.tile([128, 2048], f32, tag="o9",
                                             name="o9")
                    odst = chunk[0][:, (sp % csz) * 512:(sp % csz + 1) * 512]
                    if sp % 2:
                        nc.scalar.activation(
                            odst, pg[:], AF.Relu,
                            bias=bias3[:, oh:oh + 1],
                            scale=scale3[:, oh:oh + 1])
                    else:
                        nc.vector.tensor_scalar(
                            odst, pg[:], scale3[:, oh:oh + 1],
                            bias3[:, oh:oh + 1], op0=AL.mult, op1=AL.add)
                        nc.vector.tensor_scalar_max(odst, odst, 0.0)
                    if sp % csz == csz - 1:
                        c0 = (sp - (csz - 1)) * 512
                        nc.sync.dma_start(
                            out=out[oh, :, c0:c0 + csz * 512],
                            in_=chunk[0][:, :csz * 512])

                for i, (oh, sp) in enumerate(tiles):
                    pend.append((c3_mm(ps5, sp // 8, sp % 8, oh,
                                       stop=False), oh, sp))
                    if i == 6:
                        warm(8)
                    if i >= 6:
                        finish(*pend.pop(0))
                for args in pend:
                    finish(*args)

        psw_ctx.__exit__(None, None, None)

    nc.compile()
    return nc


# ---- host-side input prep / output assembly -----------------------------

def _prep_inputs(x, w1, g1, b1, w2, g2, b2, w3, g3, b3):
    f4 = np.float32
    h2 = np.float16
    import ml_dtypes
    bf = ml_dtypes.bfloat16
    xr = np.ascontiguousarray(x, f4).reshape(B, C, T, DHW)

    pairs = _build_pairs()
    w2r = np.ascontiguousarray(w2, f4).reshape(PL, PL, NTAP)
    blocks = []
    for (ta, tb, _lid) in pairs:
        top = np.ascontiguousarray(w2r[:, :, ta].T)           # [c, o]
        bot = (np.ascontiguousarray(w2r[:, :, tb].T) if tb is not None
               else np.zeros((PL, PL), f4))
        blocks.append(np.concatenate([top, bot], 0))          # [128, 64]
    w2t = np.concatenate(blocks, 1).astype(bf)                # [128, NPAIR*64]

    w1T = np.ascontiguousarray(w1, f4).T.reshape(2, 128, PL)  # [cb, k, o]
    w1t = np.ascontiguousarray(
        np.concatenate([w1T[0], w1T[1]], 1)).astype(h2)       # [128, 2*64]
    w3t = np.concatenate([np.ascontiguousarray(w3, f4).T] * 2, 0).copy()

    gb1 = np.stack([np.asarray(g1, f4), np.asarray(b1, f4)], 1)  # [64, 2]
    gb2 = np.stack([np.asarray(g2, f4), np.asarray(b2, f4)], 1)
    gb2 = np.concatenate([gb2, gb2], 0)
    g3r = np.asarray(g3, f4).reshape(2, 128).T
    b3r = np.asarray(b3, f4).reshape(2, 128).T
    gb3 = np.concatenate([g3r, b3r], 1).copy()  # [128,4]

    pm = np.zeros((128, NT), f4)
    for i in range(NT):
        for r in range(128):
            P = 128 * i + r
            if P < 15 * 324 + 15 * 18 + 16 and (P % 324) // 18 < 16 \
                    and P % 18 < 16:
                pm[r, i] = 1.0

    in_maps = []
    for core in range(NCORES):
        b = core // 4
        t0 = 2 * (core % 4)
        xslab = np.zeros((C, SLAB, DHW), f4)
        tm = np.zeros((SLAB,), f4)
        for si, gt in enumerate(range(t0 - 1, t0 + 3)):
            if 0 <= gt < T:
                xslab[:, si] = xr[b, :, gt]
                tm[si] = 1.0
        xs2 = xslab.reshape(2, 128, SLAB * DHW)
        in_maps.append({
            "xsb": np.ascontiguousarray(xs2).astype(h2),
            "idm": np.eye(128, dtype=h2),
            "w1t": w1t, "w2t": w2t, "w3t": w3t,
            "gb1": gb1, "gb2": gb2, "gb3": gb3,
            "tmask": np.broadcast_to(tm, (64, SLAB)).copy(),
            "pmsk": pm,
        })
    return in_maps


def kernel(x, w1, g1, b1, w2, g2, b2, w3, g3, b3):
    global LAST_RESULT
    from concourse.bass_utils import run_bass_kernel_spmd

    nc = _build(MM_DT, C2_DT)
    in_maps = _prep_inputs(x, w1, g1, b1, w2, g2, b2, w3, g3, b3)
    res = run_bass_kernel_spmd(nc, in_maps, core_ids=list(range(NCORES)))
    LAST_RESULT = res

    full = np.empty((B, C, T, D, H, W), np.float32)
    for core in range(NCORES):
        b = core // 4
        t0 = 2 * (core % 4)
        o = res.results[core]["out"].reshape(C, TPC, D, H, W)
        full[b, :, t0:t0 + TPC] = o
    return full

